# revision 20
# baseline (speedup 1.0000x reference)
"""Distributed MHA kernel for Trainium2 (8 NeuronCores, SPMD) — v4.

Problem: b=2, s=2048, e=2048, 32 heads x 64 dim, rotary_dim=32, causal,
fp32 reference.  Sharding: core c = batch*4 + head_group (tensor-parallel
over heads, data-parallel over batch).  Column-parallel Wqkv,
row-parallel Wout; the 4 partial outputs per batch are summed host-side.

Structure (all per-core, bf16 matmuls, fp32 PSUM):
  - RoPE via DVE stream_shuffle (lane i <-> i+16 per 32-lane bank) +
    fused scalar_tensor_tensor (bias+cos / bias+sin) — no DMAs.
  - Scores: K=64 head pairs at PE row-groups 0-1/2-3 run concurrently.
  - AV: M=64 head pairs at PE col-groups run concurrently; softmax
    denominators accumulate in a separate PSUM bank via M=1 matmuls
    (col positions 0/32, also concurrent).
  - exp on ACT over [128,1024] (both heads), causally sliced on
    diagonal key-tiles; [128,128] triangular mask only on the diagonal.
  - Software pipeline: AV lags scores by 2 groups; projection groups of
    the next chunk and out-projection tiles of the previous chunk fill
    tensor-engine gaps while ACT grinds exp.
"""

import numpy as np
from collections import deque

S = 2048
E = 2048
NET = 16          # e-tiles of 128
SCH = 512         # s-chunk
NCH = 4           # s-chunks


def _build_nc():
    import concourse.bacc as bacc
    import concourse.bass as bass  # noqa: F401
    import concourse.tile as tile
    from concourse import mybir

    f32 = mybir.dt.float32
    bf16 = mybir.dt.bfloat16
    AF = mybir.ActivationFunctionType
    ALU = mybir.AluOpType

    nc = bacc.Bacc(None, target_bir_lowering=False)
    xT = nc.dram_tensor("xT", [E, S], bf16, kind="ExternalInput")
    wqkv = nc.dram_tensor("wqkv", [E, 1536], bf16, kind="ExternalInput")
    wout = nc.dram_tensor("wout", [512, E], bf16, kind="ExternalInput")
    bqk = nc.dram_tensor("bqk", [128, 8], f32, kind="ExternalInput")
    bqks = nc.dram_tensor("bqks", [128, 8], f32, kind="ExternalInput")
    bv = nc.dram_tensor("bv", [128, 512], f32, kind="ExternalInput")
    crep = nc.dram_tensor("crep", [128, S], bf16, kind="ExternalInput")
    srep = nc.dram_tensor("srep", [128, S], bf16, kind="ExternalInput")
    tri = nc.dram_tensor("tri", [128, 128], bf16, kind="ExternalInput")
    y = nc.dram_tensor("y", [S, E], bf16, kind="ExternalOutput")

    swap16 = [(i + 16) % 32 for i in range(32)]

    with tile.TileContext(nc) as tc:
        from contextlib import ExitStack

        with ExitStack() as ctx:
            consts = ctx.enter_context(tc.tile_pool(name="consts", bufs=1))
            qkp = ctx.enter_context(tc.tile_pool(name="qkp", bufs=1))
            vp = ctx.enter_context(tc.tile_pool(name="vp", bufs=1))
            atp = ctx.enter_context(tc.tile_pool(name="atp", bufs=1))
            xp = ctx.enter_context(tc.tile_pool(name="xp", bufs=2))
            rtp = ctx.enter_context(tc.tile_pool(name="rtp", bufs=2))
            ptp = ctx.enter_context(tc.tile_pool(name="ptp", bufs=3))
            dnp = ctx.enter_context(tc.tile_pool(name="dnp", bufs=2))
            yp_sb = ctx.enter_context(tc.tile_pool(name="yp_sb", bufs=2))
            # PSUM: st(2x2 banks) + oT + den + rb + yp = 8 banks
            ps_s = ctx.enter_context(
                tc.tile_pool(name="ps_s", bufs=2, space="PSUM"))
            ps_o = ctx.enter_context(
                tc.tile_pool(name="ps_o", bufs=1, space="PSUM"))
            ps_m = ctx.enter_context(
                tc.tile_pool(name="ps_m", bufs=1, space="PSUM"))

            # ---- input DMAs: x chunk 0 first, then weights, then rest
            xs_t = {}

            def load_xs(tj):
                for et in range(NET):
                    t = xp.tile([128, SCH], bf16, tag=f"x{et}")
                    nc.sync.dma_start(
                        t, xT[et * 128:(et + 1) * 128,
                              tj * SCH:(tj + 1) * SCH])
                    xs_t[(et, tj)] = t

            load_xs(0)
            w_sb = []
            for et in range(NET):
                t = consts.tile([128, 1536], bf16, tag=f"w{et}")
                nc.gpsimd.dma_start(t, wqkv[et * 128:(et + 1) * 128, :])
                w_sb.append(t)
            crep_sb = consts.tile([128, S], bf16, tag="crep")
            nc.scalar.dma_start(crep_sb, crep[:, :])
            srep_sb = consts.tile([128, S], bf16, tag="srep")
            nc.scalar.dma_start(srep_sb, srep[:, :])
            bqk_sb = consts.tile([128, 8], f32, tag="bqk")
            nc.sync.dma_start(bqk_sb, bqk[:, :])
            bqks_sb = consts.tile([128, 8], f32, tag="bqks")
            nc.sync.dma_start(bqks_sb, bqks[:, :])
            bv_sb = consts.tile([128, 512], f32, tag="bv")
            nc.sync.dma_start(bv_sb, bv[:, :])
            tri_sb = consts.tile([128, 128], bf16, tag="tri")
            nc.sync.dma_start(tri_sb, tri[:, :])
            wo_sb = []
            for pr in range(4):
                t = consts.tile([128, E], bf16, tag=f"wo{pr}")
                nc.gpsimd.dma_start(t, wout[pr * 128:(pr + 1) * 128, :])
                wo_sb.append(t)
            ones_sb = consts.tile([1, 64], bf16, tag="ones")
            nc.vector.memset(ones_sb, 1.0)
            ones128 = consts.tile([128, 1], bf16, tag="ones128")
            nc.vector.memset(ones128, 1.0)

            qk_t = {}
            v_t = {}
            at_t = {}

            # ---- phase A group emitters (qk proj + rope / v proj).
            # They use the ps_m pool (alternating rb/yp tags) so they can
            # run as filler during phase B.
            def make_qk_group(tj, ft, tag):
                def emit():
                    ps = ps_m.tile([128, 512], f32, tag=tag)
                    for et in range(NET):
                        nc.tensor.matmul(
                            ps,
                            lhsT=w_sb[et][:, ft * 128:(ft + 1) * 128],
                            rhs=xs_t[(et, tj)],
                            start=(et == 0), stop=(et == NET - 1))
                    cs = slice(tj * SCH, (tj + 1) * SCH)
                    tmp = rtp.tile([128, SCH], f32, tag="rtmp")
                    nc.vector.stream_shuffle(tmp, ps, swap16)
                    qt = qkp.tile([128, SCH], bf16, tag=f"q{ft}_{tj}")
                    nc.vector.scalar_tensor_tensor(
                        qt, ps, bqk_sb[:, ft:ft + 1], crep_sb[:, cs],
                        op0=ALU.add, op1=ALU.mult)
                    ts = rtp.tile([128, SCH], bf16, tag="rts", bufs=1)
                    nc.vector.scalar_tensor_tensor(
                        ts, tmp, bqks_sb[:, ft:ft + 1], srep_sb[:, cs],
                        op0=ALU.add, op1=ALU.mult)
                    nc.vector.tensor_add(qt, qt, ts)
                    qk_t[(ft, tj)] = qt
                return emit

            def make_v_group(tj, us, tag):
                def emit():
                    ut = tj * 4 + us
                    ps = ps_m.tile([128, 512], f32, tag=tag)
                    for et in range(NET):
                        nc.tensor.matmul(
                            ps,
                            lhsT=xs_t[(et, tj)][:, us * 128:(us + 1) * 128],
                            rhs=w_sb[et][:, 1024:1536],
                            start=(et == 0), stop=(et == NET - 1))
                    vt = vp.tile([128, 512], bf16, tag=f"v{ut}")
                    nc.vector.tensor_add(vt, ps, bv_sb)
                    v_t[ut] = vt
                return emit

            def a_units(tj):
                units = []
                for i, ft in enumerate(range(8)):
                    units.append(
                        make_qk_group(tj, ft, "rb" if i % 2 else "yp"))
                for us in range(4):
                    units.append(
                        make_v_group(tj, us, "rb" if us % 2 else "yp"))
                return units

            # ---- phase C tile emitter (filler for the next chunk)
            drain = [False]
            dctr = [0]

            def make_c_tile(tj, tt, ec):
                def emit():
                    i = dctr[0]
                    dctr[0] += 1
                    tag = "rb" if (drain[0] and i % 2) else "yp"
                    yp = ps_m.tile([128, 512], f32, tag=tag)
                    for pr in range(4):
                        nc.tensor.matmul(
                            yp,
                            lhsT=at_t[(pr, tj)][:, tt * 128:(tt + 1) * 128],
                            rhs=wo_sb[pr][:, ec * 512:(ec + 1) * 512],
                            start=(pr == 0), stop=(pr == 3))
                    ys = yp_sb.tile([128, 512], bf16, tag="ys")
                    if drain[0] and i % 2:
                        nc.scalar.copy(ys, yp)
                    else:
                        nc.vector.tensor_copy(ys, yp)
                    tt_abs = tj * 4 + tt
                    nc.sync.dma_start(
                        y[tt_abs * 128:(tt_abs + 1) * 128,
                          ec * 512:(ec + 1) * 512], ys)
                return emit

            # ---- filler machinery (A-groups and C-tiles + chunk marks)
            fillers = deque()
            popped_marks = set()

            def pop_filler():
                while fillers:
                    f = fillers.popleft()
                    if callable(f):
                        f()
                        return
                    popped_marks.add(f)

            def flush_to(mark):
                while mark not in popped_marks and fillers:
                    f = fillers.popleft()
                    if callable(f):
                        f()
                    else:
                        popped_marks.add(f)

            marks = [f"A{t}done" for t in range(NCH + 1)]

            # ---- deferred tail of the softmax normalization
            pending_norm = deque()

            def make_norm_post(pr, tj, oT, rcb):
                def emit():
                    rb_ps = ps_m.tile([128, 512], f32, tag="rb")
                    nc.tensor.matmul(rb_ps[0:64, :], lhsT=ones_sb,
                                     rhs=rcb[:, 0:512],
                                     start=True, stop=True)
                    nc.tensor.matmul(rb_ps[64:128, :], lhsT=ones_sb,
                                     rhs=rcb[:, 512:1024],
                                     start=True, stop=True)
                    rb_sb = dnp.tile([128, 512], bf16, tag="rbsb")
                    nc.vector.tensor_copy(rb_sb, rb_ps)
                    at = atp.tile([128, 512], bf16, tag=f"at{pr}_{tj}")
                    nc.vector.tensor_mul(at, oT, rb_sb)
                    at_t[(pr, tj)] = at
                return emit

            # ================= emission =================
            for u in a_units(0):
                u()
            load_xs(1)
            fillers.extend(a_units(1))
            fillers.append(marks[1])

            for tj in range(NCH):
                nu = 4 * tj + 4
                if tj >= 1:
                    flush_to(marks[tj])
                    if tj + 1 < NCH:
                        fillers.extend(a_units(tj + 1))
                        fillers.append(marks[tj + 1])
                if tj + 2 < NCH:
                    load_xs(tj + 2)
                for pr in range(4):
                    oT = ps_o.tile([128, 512], f32, tag="oT")
                    den = ps_o.tile([128, 512], f32, tag="den")
                    qtile = qk_t[(pr, tj)]
                    pend = deque()

                    def emit_av(ut, pt, dlo):
                        # has_written is per-element: each writer's
                        # first matmul uses start=True for its own
                        # partition range of the shared bank.
                        h0 = 128 * pr
                        nc.tensor.matmul(
                            oT[0:64, dlo:512],
                            lhsT=v_t[ut][:, h0:h0 + 64],
                            rhs=pt[:, dlo:512],
                            start=(ut == 0), stop=(ut == nu - 1))
                        nc.tensor.matmul(
                            oT[64:128, dlo:512],
                            lhsT=v_t[ut][:, h0 + 64:h0 + 128],
                            rhs=pt[:, 512 + dlo:1024],
                            start=(ut == 0), stop=(ut == nu - 1),
                            skip_group_check=True)
                        nc.tensor.matmul(
                            den[0:1, dlo:512], lhsT=ones128,
                            rhs=pt[:, dlo:512],
                            start=(ut == 0), stop=(ut == nu - 1))
                        nc.tensor.matmul(
                            den[32:33, dlo:512], lhsT=ones128,
                            rhs=pt[:, 512 + dlo:1024],
                            start=(ut == 0), stop=(ut == nu - 1),
                            skip_group_check=True)

                    for ut in range(nu):
                        # ready AV work first — a score matmul may stall
                        # on exp (in-order tensor queue)
                        if ut >= 2 and pend:
                            if pending_norm:
                                pending_norm.popleft()()
                            emit_av(*pend.popleft())
                            if ut % 2 == 0:
                                pop_filler()
                        jj, us = divmod(ut, 4)
                        diag = ut >= 4 * tj
                        dlo = 128 * (ut - 4 * tj) if diag else 0
                        st = ps_s.tile([128, 1024], f32, tag="st")
                        kt = qk_t[(4 + pr, jj)]
                        usl = slice(us * 128, (us + 1) * 128)
                        nc.tensor.matmul(
                            st[:, dlo:512],
                            lhsT=kt[0:64, usl], rhs=qtile[0:64, dlo:512],
                            start=True, stop=True)
                        nc.tensor.matmul(
                            st[:, 512 + dlo:1024],
                            lhsT=kt[64:128, usl], rhs=qtile[64:128, dlo:512],
                            start=True, stop=True)
                        pt = ptp.tile([128, 1024], bf16, tag="pt")
                        if dlo == 0:
                            nc.scalar.activation(pt, st, AF.Exp, scale=0.125)
                        else:
                            st3 = st.rearrange("p (h q) -> p h q", h=2)
                            pt3 = pt.rearrange("p (h q) -> p h q", h=2)
                            nc.scalar.activation(
                                pt3[:, :, dlo:512], st3[:, :, dlo:512],
                                AF.Exp, scale=0.125)
                        if diag:
                            nc.vector.tensor_mul(
                                pt[:, dlo:dlo + 128],
                                pt[:, dlo:dlo + 128], tri_sb)
                            nc.vector.tensor_mul(
                                pt[:, 512 + dlo:640 + dlo],
                                pt[:, 512 + dlo:640 + dlo], tri_sb)
                        pend.append((ut, pt, dlo))
                        if ut == 0:
                            pop_filler()
                    while pend:
                        if pending_norm:
                            pending_norm.popleft()()
                        emit_av(*pend.popleft())
                    # normalization head: reciprocal of the denominators
                    # (rows 0 / 32 of den; custom-DVE recip needs its
                    # input at partition 0)
                    dn = dnp.tile([1, 1024], f32, tag="dn", bufs=1)
                    nc.vector.tensor_copy(dn[:, 0:512], den[0:1, :])
                    nc.vector.tensor_copy(dn[:, 512:1024], den[32:33, :])
                    rc = dnp.tile([1, 1024], f32, tag="rc", bufs=1)
                    nc.vector.reciprocal_approx_fast(out=rc, in_=dn)
                    rcb = dnp.tile([1, 1024], bf16, tag="rcb")
                    nc.vector.tensor_copy(rcb, rc)
                    pending_norm.append(make_norm_post(pr, tj, oT, rcb))
                # flush the chunk's last norm before its C tiles queue up
                while pending_norm:
                    pending_norm.popleft()()
                for tt in range(4):
                    for ec in range(4):
                        fillers.append(make_c_tile(tj, tt, ec))
            drain[0] = True
            while fillers:
                f = fillers.popleft()
                if callable(f):
                    f()
    nc.compile()
    return nc


_CACHE = {}


def _host_consts():
    import ml_dtypes
    bf = ml_dtypes.bfloat16
    inv = 1.0 / (10000.0 ** (np.arange(0, 32, 2, dtype=np.float64) / 32.0))
    t = np.arange(S, dtype=np.float64)
    fr = np.outer(t, inv)                       # [s, 16]
    cos = np.cos(fr).astype(np.float32).T       # [16, s]
    sin = np.sin(fr).astype(np.float32).T
    crep = np.ones((128, S), np.float32)
    srep = np.zeros((128, S), np.float32)
    for blk in (0, 64):
        crep[blk:blk + 16] = cos
        crep[blk + 16:blk + 32] = cos
        srep[blk:blk + 16] = -sin
        srep[blk + 16:blk + 32] = sin
    ui = np.arange(128)[:, None]
    ci = np.arange(128)[None, :]
    tri = (ui <= ci).astype(np.float32)
    return crep.astype(bf), srep.astype(bf), tri.astype(bf)


def kernel(**inputs):
    import ml_dtypes
    from concourse.bass_utils import run_bass_kernel_spmd

    x = np.asarray(inputs["x"], np.float32)
    Wqkv = np.asarray(inputs["Wqkv"], np.float32)
    bqkv = np.asarray(inputs["bqkv"], np.float32)
    Wout = np.asarray(inputs["Wout"], np.float32)
    bout = np.asarray(inputs["bout"], np.float32)

    if "nc" not in _CACHE:
        _CACHE["nc"] = _build_nc()
    nc = _CACHE["nc"]

    bf = ml_dtypes.bfloat16
    crep, srep, tri = _host_consts()
    r = np.arange(128)
    perm = (r // 32) * 32 + ((r % 32) + 16) % 32
    in_maps = []
    for c in range(8):
        b, g = divmod(c, 4)
        gs = slice(g * 512, (g + 1) * 512)
        wq = Wqkv[:, 0:2048][:, gs]
        wk = Wqkv[:, 2048:4096][:, gs]
        wv = Wqkv[:, 4096:6144][:, gs]
        bq = bqkv[0:2048][gs]
        bk = bqkv[2048:4096][gs]
        bvv = bqkv[4096:6144][gs]
        bqk_c = np.concatenate([bq, bk]).reshape(8, 128).T.astype(
            np.float32).copy()
        in_maps.append(dict(
            xT=np.ascontiguousarray(x[b].T).astype(bf),
            wqkv=np.concatenate([wq, wk, wv], axis=1).astype(bf),
            wout=Wout[gs, :].astype(bf),
            bqk=bqk_c,
            bqks=np.ascontiguousarray(bqk_c[perm, :]),
            bv=np.broadcast_to(
                bvv.astype(np.float32), (128, 512)).copy(),
            crep=crep, srep=srep, tri=tri,
        ))
    kwargs = _CACHE.get("run_kwargs", {})
    res = run_bass_kernel_spmd(nc, in_maps, list(range(8)), **kwargs)
    _CACHE["last_results"] = res
    out = np.zeros((2, S, E), np.float32)
    for c in range(8):
        out[c // 4] += res.results[c]["y"].astype(np.float32)
    out += bout[None, None, :]
    return out
